# revision 14
# baseline (speedup 1.0000x reference)
"""DEMA (double exponential smoothing) Trainium2 Bass kernel.

Math
----
Reference recurrence (per batch b, channel c, over time t):
    s0 = x[0], b0 = x[1] - x[0]
    s_t = a*x_t + (1-a)*(s_{t-1} + b_{t-1})
    b_t = bt*(s_t - s_{t-1}) + (1-bt)*b_{t-1}
    out = [s0, s_1, ..., s_{T-1}]

With state z = [s, b]: z_t = M z_{t-1} + v x_t where
    M = [[1-a, 1-a], [-a*bt, 1-a*bt]],  v = [a, a*bt]
and the first two outputs are exact copies: out_0 = x_0, out_1 = x_1,
with z_1 = [x_1, x_1 - x_0].

Algorithm: one 128x128 fp16 matmul per 126-step time block. Each
batch's rhs is a single persistent SBUF tile [128, 33*512]: partitions
2..127 hold the 33 time blocks column-chunk-wise (host pre-transposes x
into [126, 33, 512] so the load is one DMA with 126 contiguous 33 KiB
descriptors); partitions 0..1 of chunk j hold the block's input state
(chunk 0: x_0/x_1 via a tiny DMA; chunks 1..32: cast-copied from PSUM
rows 0..1 of the previous block's matmul). The constant lhsT
(host-built in float64 from runtime alpha/beta) maps chunk j to
[s_out; b_out; s_t0..t0+125]; PSUM is evicted whole (128 rows) into
fp16 out tiles whose rows 0..1 are dead, and y returns host-side from a
transposed DRAM layout [126, 34, 512] (chunk 33 carries y rows 0..1).
The 4 per-core batch chains interleave on the TensorEngine so the
matmul -> state-copy -> matmul serial chain hides behind 4x of
independent matmul work; state copies round-robin over
gpsimd/scalar/vector, evictions alternate scalar/vector, input DMAs
issue from sync and output DMAs from sync as well.

Everything is fp16 end-to-end (PSUM accumulates fp32). Correctness
gate is rel 2e-2; the fp16 pipeline measures ~3.7e-4 across the full
(alpha, beta) range. Per-core traffic 33.5 MB at ~360 GB/s -> ~95 us
roofline.

Sharding: batch 32 -> 4 per core across 8 cores (data parallel; the
recurrence is independent per (b, c)).
"""

import numpy as np

import concourse.bacc as bacc
import concourse.bass as bass
import concourse.mybir as mybir
from concourse import tile
from concourse.bass_utils import run_bass_kernel_spmd

N_CORES = 8
P = 128            # SBUF partitions
B, T, C = 32, 4096, 512
BC = B // N_CORES  # batches per core
L = 126            # time steps per block (rhs rows 2..127)
NBLK = 32          # full blocks; plus one 62-row tail block
TAIL = T - 2 - L * NBLK  # 62
NCH = NBLK + 1     # 33 rhs chunks
OMEGA = 8          # blocks per out mega tile

_F32 = mybir.dt.float32
_MM_DT = mybir.dt.float16
_NP_MM = np.float16


def _host_weights(a: float, bt: float):
    """Build [2, 128, 128] lhsT weights (W_first, W_mid) in float64->fp16.

    lhsT[k, m]: k = rhs row (0=s_in, 1=b_in, 2+l = x_l),
                m = out row (0=s_out, 1=b_out, 2+t = s at local t).
    """
    M = np.array([[1 - a, 1 - a], [-a * bt, 1 - a * bt]])
    v = np.array([a, a * bt])
    g = np.zeros(L)
    h = np.zeros(L)
    cur = v.copy()
    for j in range(L):
        g[j], h[j] = cur
        cur = M @ cur
    Mp = np.zeros((L, 2, 2))  # Mp[t] = M^(t+1)
    acc = np.eye(2)
    for t in range(L):
        acc = acc @ M
        Mp[t] = acc
    ML = Mp[L - 1]  # M^L

    W = np.zeros((P, P))
    W[0, 0], W[1, 0] = ML[0, 0], ML[0, 1]
    W[0, 1], W[1, 1] = ML[1, 0], ML[1, 1]
    W[0, 2:] = Mp[:, 0, 0]
    W[1, 2:] = Mp[:, 0, 1]
    for l in range(L):
        W[2 + l, 0] = g[L - 1 - l]
        W[2 + l, 1] = h[L - 1 - l]
        W[2 + l, 2 + l : 2 + L] = g[: L - l]
    Wf = np.zeros((P, P))
    Wf[0, :] = -W[1, :]
    Wf[1, :] = W[0, :] + W[1, :]
    Wf[2:, :] = W[2:, :]
    return np.stack([Wf, W]).astype(_NP_MM)


def _build(bcount=BC, c_len=C):
    nc = bacc.Bacc("TRN2", target_bir_lowering=False, debug=False)
    # xt: host-transposed input, xt[b, p, j, :] = x[b, 2 + 126j + p, :]
    # (p in 0..125; tail chunk 32 zero-padded past p=61)
    xt = nc.dram_tensor("xt", [bcount, L, NCH, c_len], _MM_DT, kind="ExternalInput")
    x01 = nc.dram_tensor("x01", [bcount, 2, c_len], _MM_DT, kind="ExternalInput")
    wd = nc.dram_tensor("wts", [2, P, P], _MM_DT, kind="ExternalInput")
    # yt: transposed output, yt[b, t, j, :] = y row 2+126j+t (j<33);
    # chunk 33 rows 0..1 = y rows 0..1
    yt = nc.dram_tensor("yt", [bcount, L, NCH + 1, c_len], _MM_DT, kind="ExternalOutput")

    # input sub-DMA chunk ranges: small first chunks for fast rampup
    SUBS = [(0, 4), (4, 8), (8, 16), (16, 24), (24, 32), (32, 33)]

    with tile.TileContext(nc) as tc:
        with (
            tc.tile_pool(name="wpool", bufs=1) as wpool,
            tc.tile_pool(name="inpool", bufs=bcount) as inpool,
            tc.tile_pool(name="ompool", bufs=5) as ompool,
            tc.tile_pool(name="tailpool", bufs=bcount) as tailpool,
            tc.tile_pool(name="psum", bufs=8, space="PSUM") as pspool,
        ):
            wt = wpool.tile([P, 2 * P], _MM_DT)
            nc.sync.dma_start(
                wt[:].rearrange("k (m t) -> k m t", m=2),
                wd[:].rearrange("m k t -> k m t"),
            )

            giant = {}
            for b in range(bcount):
                giant[b] = inpool.tile(
                    [P, NCH * c_len], _MM_DT, tag="giant", name=f"giant_{b}"
                )
                nc.sync.dma_start(giant[b][0:2, 0:c_len], x01[b])
            for lo, hi in SUBS:
                for b in range(bcount):
                    nc.sync.dma_start(
                        giant[b][2:P, lo * c_len : hi * c_len],
                        xt[b, :, lo:hi, :].rearrange("p j c -> p (j c)"),
                    )

            om: dict = {}
            omt: dict = {}
            for j in range(NCH):
                for b in range(bcount):
                    if j % OMEGA == 0 and j < NBLK:
                        om[(b, j // OMEGA)] = ompool.tile(
                            [P, OMEGA * c_len], _MM_DT, tag="om",
                            name=f"om_{b}_{j // OMEGA}",
                        )
                    if j == NBLK:
                        omt[b] = tailpool.tile(
                            [P, c_len], _MM_DT, tag="omt", name=f"omt_{b}"
                        )

                    ps = pspool.tile([P, c_len], _F32, tag="ps")
                    if j == 0:
                        nc.tensor.matmul(
                            ps[:], wt[:, 0:P], giant[b][:, 0:c_len],
                            start=True, stop=True,
                        )
                    elif j < NBLK:
                        nc.tensor.matmul(
                            ps[:],
                            wt[:, P : 2 * P],
                            giant[b][:, j * c_len : (j + 1) * c_len],
                            start=True,
                            stop=True,
                        )
                    else:
                        nc.tensor.matmul(
                            ps[:],
                            wt[0:64, P : 2 * P],
                            giant[b][0:64, j * c_len : (j + 1) * c_len],
                            start=True,
                            stop=True,
                        )

                    # Evict full PSUM (rows 0..1 ride along; dead in om) and
                    # thread the state: PSUM rows 0..1 -> rhs rows 0..1 of
                    # chunk j+1. GPSIMD cannot read PSUM, so even blocks copy
                    # state from PSUM on whichever of scalar/vector is NOT
                    # evicting (parallel, short chain) and odd blocks copy
                    # fp16 state out of the freshly evicted om tile on gpsimd
                    # (keeps scalar/vector load bounded).
                    if j < NBLK:
                        edst = om[(b, j // OMEGA)][
                            :, (j % OMEGA) * c_len : (j % OMEGA + 1) * c_len
                        ]
                        esrc = ps[:]
                    else:
                        edst = omt[b][0 : 2 + TAIL, :]
                        esrc = ps[0 : 2 + TAIL, :]
                    vec_evicts = (j * bcount + b) % 2 == 0
                    if j < NBLK:
                        sdst = giant[b][0:2, (j + 1) * c_len : (j + 2) * c_len]
                    if j < NBLK and j % 2 == 0:
                        if vec_evicts:
                            nc.scalar.copy(sdst, ps[0:2, :])
                        else:
                            nc.vector.tensor_copy(sdst, ps[0:2, :])
                    if vec_evicts:
                        nc.vector.tensor_copy(edst, esrc)
                    else:
                        nc.scalar.copy(edst, esrc)
                    if j < NBLK and j % 2 == 1:
                        nc.gpsimd.tensor_copy(
                            sdst,
                            om[(b, j // OMEGA)][
                                0:2, (j % OMEGA) * c_len : (j % OMEGA) * c_len + c_len
                            ],
                        )

                    if j == 0:
                        # y rows 0..1 = x rows 0..1, staged in yt chunk 33
                        nc.sync.dma_start(
                            yt[b, 0:2, NCH, :], giant[b][0:2, 0:c_len]
                        )
                    if j < NBLK and j % OMEGA == OMEGA - 1:
                        m = j // OMEGA
                        nc.sync.dma_start(
                            yt[b, :, m * OMEGA : (m + 1) * OMEGA, :],
                            om[(b, m)][2:P].rearrange(
                                "p (jj c) -> p jj c", jj=OMEGA
                            ),
                        )
                    if j == NBLK:
                        nc.sync.dma_start(
                            yt[b, 0:TAIL, NBLK, :], omt[b][2 : 2 + TAIL, :]
                        )
    nc.compile()
    return nc


_MODULE_CACHE: dict = {}


def _get_module(**kw):
    key = tuple(sorted(kw.items()))
    if key not in _MODULE_CACHE:
        _MODULE_CACHE[key] = _build(**kw)
    return _MODULE_CACHE[key]


def make_in_maps(x, alpha, beta, bcount=BC, n_cores=N_CORES):
    a = float(np.asarray(alpha).reshape(-1)[0])
    bt = float(np.asarray(beta).reshape(-1)[0])
    wts = _host_weights(a, bt)
    in_maps = []
    for i in range(n_cores):
        xs = np.asarray(x[i * bcount : (i + 1) * bcount], dtype=_NP_MM)
        body = np.zeros((bcount, NCH * L, C), dtype=_NP_MM)
        body[:, : T - 2] = xs[:, 2:]
        xt = np.ascontiguousarray(
            body.reshape(bcount, NCH, L, C).transpose(0, 2, 1, 3)
        )
        x01 = np.ascontiguousarray(xs[:, 0:2])
        in_maps.append({"xt": xt, "x01": x01, "wts": wts})
    return in_maps


def _gather(res, n_cores=N_CORES, bcount=BC):
    out = np.empty((B, T, C), dtype=np.float32)
    for i in range(n_cores):
        ytc = res.results[i]["yt"]  # [bcount, L, NCH+1, C] fp16
        gb = i * bcount
        out[gb : gb + bcount, 0:2] = ytc[:, 0:2, NCH]
        body = ytc[:, :, 0:NBLK].transpose(0, 2, 1, 3).reshape(bcount, NBLK * L, C)
        out[gb : gb + bcount, 2 : 2 + NBLK * L] = body
        out[gb : gb + bcount, 2 + NBLK * L :] = ytc[:, 0:TAIL, NBLK]
    return out


def _run(x, alpha, beta, trace=False, **kw):
    x = np.asarray(x, dtype=np.float32)
    assert x.shape == (B, T, C), x.shape
    in_maps = make_in_maps(x, alpha, beta)
    nc = _get_module()
    res = run_bass_kernel_spmd(nc, in_maps, list(range(N_CORES)), trace=trace, **kw)
    return _gather(res), res


def kernel(x, alpha, beta):
    return _run(x, alpha, beta)[0]


# revision 16
# speedup vs baseline: 1.4671x; 1.4671x over previous
"""DEMA (double exponential smoothing) Trainium2 Bass kernel.

Math
----
Reference recurrence (per batch b, channel c, over time t):
    s0 = x[0], b0 = x[1] - x[0]
    s_t = a*x_t + (1-a)*(s_{t-1} + b_{t-1})
    b_t = bt*(s_t - s_{t-1}) + (1-bt)*b_{t-1}
    out = [s0, s_1, ..., s_{T-1}]

With state z = [s, b]: z_t = M z_{t-1} + v x_t where
    M = [[1-a, 1-a], [-a*bt, 1-a*bt]],  v = [a, a*bt]
and the first two outputs are exact copies: out_0 = x_0, out_1 = x_1,
with z_1 = [x_1, x_1 - x_0].

Algorithm: one 128x128 fp16 matmul per 126-step time block. Each
batch's rhs is a single persistent SBUF tile [128, 33*512]: partitions
2..127 hold the 33 time blocks column-chunk-wise (host pre-transposes x
into [126, 33, 512] so the load is one DMA with 126 contiguous 33 KiB
descriptors); partitions 0..1 of chunk j hold the block's input state
(chunk 0: x_0/x_1 via a tiny DMA; chunks 1..32: cast-copied from PSUM
rows 0..1 of the previous block's matmul). The constant lhsT
(host-built in float64 from runtime alpha/beta) maps chunk j to
[s_out; b_out; s_t0..t0+125]; PSUM is evicted whole (128 rows) into
fp16 out tiles whose rows 0..1 are dead, and y returns host-side from a
transposed DRAM layout [126, 34, 512] (chunk 33 carries y rows 0..1).
The 4 per-core batch chains interleave on the TensorEngine so the
matmul -> state-copy -> matmul serial chain hides behind 4x of
independent matmul work; state copies round-robin over
gpsimd/scalar/vector, evictions alternate scalar/vector, input DMAs
issue from sync and output DMAs from sync as well.

Everything is fp16 end-to-end (PSUM accumulates fp32). Correctness
gate is rel 2e-2; the fp16 pipeline measures ~3.7e-4 across the full
(alpha, beta) range. Per-core traffic 33.5 MB at ~360 GB/s -> ~95 us
roofline.

Sharding: batch 32 -> 4 per core across 8 cores (data parallel; the
recurrence is independent per (b, c)).
"""

import numpy as np

import concourse.bacc as bacc
import concourse.bass as bass
import concourse.mybir as mybir
from concourse import tile
from concourse.bass_utils import run_bass_kernel_spmd

N_CORES = 8
P = 128            # SBUF partitions
B, T, C = 32, 4096, 512
BC = B // N_CORES  # batches per core
L = 126            # time steps per block (rhs rows 2..127)
NBLK = 32          # full blocks; plus one 62-row tail block
TAIL = T - 2 - L * NBLK  # 62
NCH = NBLK + 1     # 33 rhs chunks
OMEGA = 8          # blocks per out mega tile

_F32 = mybir.dt.float32
_MM_DT = mybir.dt.float16
_NP_MM = np.float16


def _host_weights(a: float, bt: float):
    """Build [2, 128, 128] lhsT weights (W_first, W_mid) in float64->fp16.

    lhsT[k, m]: k = rhs row (0=s_in, 1=b_in, 2+l = x_l),
                m = out row (0=s_out, 1=b_out, 2+t = s at local t).
    """
    M = np.array([[1 - a, 1 - a], [-a * bt, 1 - a * bt]])
    v = np.array([a, a * bt])
    g = np.zeros(L)
    h = np.zeros(L)
    cur = v.copy()
    for j in range(L):
        g[j], h[j] = cur
        cur = M @ cur
    Mp = np.zeros((L, 2, 2))  # Mp[t] = M^(t+1)
    acc = np.eye(2)
    for t in range(L):
        acc = acc @ M
        Mp[t] = acc
    ML = Mp[L - 1]  # M^L

    W = np.zeros((P, P))
    W[0, 0], W[1, 0] = ML[0, 0], ML[0, 1]
    W[0, 1], W[1, 1] = ML[1, 0], ML[1, 1]
    W[0, 2:] = Mp[:, 0, 0]
    W[1, 2:] = Mp[:, 0, 1]
    for l in range(L):
        W[2 + l, 0] = g[L - 1 - l]
        W[2 + l, 1] = h[L - 1 - l]
        W[2 + l, 2 + l : 2 + L] = g[: L - l]
    Wf = np.zeros((P, P))
    Wf[0, :] = -W[1, :]
    Wf[1, :] = W[0, :] + W[1, :]
    Wf[2:, :] = W[2:, :]
    return np.stack([Wf, W]).astype(_NP_MM)


def _build(bcount=BC, c_len=C):
    nc = bacc.Bacc("TRN2", target_bir_lowering=False, debug=False)
    # xt: host-transposed input, xt[b, p, j, :] = x[b, 2 + 126j + p, :]
    # (p in 0..125; tail chunk 32 zero-padded past p=61)
    xt = nc.dram_tensor("xt", [bcount, L, NCH, c_len], _MM_DT, kind="ExternalInput")
    x01 = nc.dram_tensor("x01", [bcount, 2, c_len], _MM_DT, kind="ExternalInput")
    wd = nc.dram_tensor("wts", [2, P, P], _MM_DT, kind="ExternalInput")
    # yt: transposed output, yt[b, t, j, :] = y row 2+126j+t (j<33);
    # chunk 33 rows 0..1 = y rows 0..1
    yt = nc.dram_tensor("yt", [bcount, L, NCH + 1, c_len], _MM_DT, kind="ExternalOutput")

    # input sub-DMA chunk ranges: small first chunks for fast rampup
    SUBS = [(0, 4), (4, 8), (8, 16), (16, 24), (24, 32), (32, 33)]

    with tile.TileContext(nc) as tc:
        with (
            tc.tile_pool(name="wpool", bufs=1) as wpool,
            tc.tile_pool(name="inpool", bufs=bcount) as inpool,
            tc.tile_pool(name="ompool", bufs=5) as ompool,
            tc.tile_pool(name="tailpool", bufs=bcount) as tailpool,
            tc.tile_pool(name="psum", bufs=8, space="PSUM") as pspool,
        ):
            wt = wpool.tile([P, 2 * P], _MM_DT)
            nc.sync.dma_start(
                wt[:].rearrange("k (m t) -> k m t", m=2),
                wd[:].rearrange("m k t -> k m t"),
            )

            giant = {}
            for b in range(bcount):
                giant[b] = inpool.tile(
                    [P, NCH * c_len], _MM_DT, tag="giant", name=f"giant_{b}"
                )
                nc.sync.dma_start(giant[b][0:2, 0:c_len], x01[b])
            for lo, hi in SUBS:
                for b in range(bcount):
                    nc.sync.dma_start(
                        giant[b][2:P, lo * c_len : hi * c_len],
                        xt[b, :, lo:hi, :].rearrange("p j c -> p (j c)"),
                    )

            om: dict = {}
            omt: dict = {}
            for j in range(NCH):
                for b in range(bcount):
                    if j % OMEGA == 0 and j < NBLK:
                        om[(b, j // OMEGA)] = ompool.tile(
                            [P, OMEGA * c_len], _MM_DT, tag="om",
                            name=f"om_{b}_{j // OMEGA}",
                        )
                    if j == NBLK:
                        omt[b] = tailpool.tile(
                            [P, c_len], _MM_DT, tag="omt", name=f"omt_{b}"
                        )

                    ps = pspool.tile([P, c_len], _F32, tag="ps")
                    if j == 0:
                        nc.tensor.matmul(
                            ps[:], wt[:, 0:P], giant[b][:, 0:c_len],
                            start=True, stop=True,
                        )
                    elif j < NBLK:
                        nc.tensor.matmul(
                            ps[:],
                            wt[:, P : 2 * P],
                            giant[b][:, j * c_len : (j + 1) * c_len],
                            start=True,
                            stop=True,
                        )
                    else:
                        nc.tensor.matmul(
                            ps[:],
                            wt[0:64, P : 2 * P],
                            giant[b][0:64, j * c_len : (j + 1) * c_len],
                            start=True,
                            stop=True,
                        )

                    # Thread the state (PSUM rows 0..1 -> rhs rows 0..1 of
                    # chunk j+1) on whichever of scalar/vector is NOT doing
                    # this block's eviction, so the chain-critical copy runs
                    # in parallel with the eviction. GPSIMD cannot touch PSUM
                    # and its SBUF copies measure ~1.9us, so it only issues
                    # the output DMAs (separate DMA queue from sync's input
                    # stream).
                    if j < NBLK:
                        edst = om[(b, j // OMEGA)][
                            :, (j % OMEGA) * c_len : (j % OMEGA + 1) * c_len
                        ]
                        esrc = ps[:]
                    else:
                        edst = omt[b][0 : 2 + TAIL, :]
                        esrc = ps[0 : 2 + TAIL, :]
                    vec_evicts = (j * bcount + b) % 2 == 0
                    if j < NBLK:
                        sdst = giant[b][0:2, (j + 1) * c_len : (j + 2) * c_len]
                        if vec_evicts:
                            nc.scalar.copy(sdst, ps[0:2, :])
                        else:
                            nc.vector.tensor_copy(sdst, ps[0:2, :])
                    if vec_evicts:
                        nc.vector.tensor_copy(edst, esrc)
                    else:
                        nc.scalar.copy(edst, esrc)

                    if j == 0:
                        # y rows 0..1 = x rows 0..1, staged in yt chunk 33
                        nc.gpsimd.dma_start(
                            yt[b, 0:2, NCH, :], giant[b][0:2, 0:c_len]
                        )
                    if j < NBLK and j % OMEGA == OMEGA - 1:
                        m = j // OMEGA
                        nc.gpsimd.dma_start(
                            yt[b, :, m * OMEGA : (m + 1) * OMEGA, :],
                            om[(b, m)][2:P].rearrange(
                                "p (jj c) -> p jj c", jj=OMEGA
                            ),
                        )
                    if j == NBLK:
                        nc.gpsimd.dma_start(
                            yt[b, 0:TAIL, NBLK, :], omt[b][2 : 2 + TAIL, :]
                        )
    nc.compile()
    return nc


_MODULE_CACHE: dict = {}


def _get_module(**kw):
    key = tuple(sorted(kw.items()))
    if key not in _MODULE_CACHE:
        _MODULE_CACHE[key] = _build(**kw)
    return _MODULE_CACHE[key]


def make_in_maps(x, alpha, beta, bcount=BC, n_cores=N_CORES):
    a = float(np.asarray(alpha).reshape(-1)[0])
    bt = float(np.asarray(beta).reshape(-1)[0])
    wts = _host_weights(a, bt)
    in_maps = []
    for i in range(n_cores):
        xs = np.asarray(x[i * bcount : (i + 1) * bcount], dtype=_NP_MM)
        body = np.zeros((bcount, NCH * L, C), dtype=_NP_MM)
        body[:, : T - 2] = xs[:, 2:]
        xt = np.ascontiguousarray(
            body.reshape(bcount, NCH, L, C).transpose(0, 2, 1, 3)
        )
        x01 = np.ascontiguousarray(xs[:, 0:2])
        in_maps.append({"xt": xt, "x01": x01, "wts": wts})
    return in_maps


def _gather(res, n_cores=N_CORES, bcount=BC):
    out = np.empty((B, T, C), dtype=np.float32)
    for i in range(n_cores):
        ytc = res.results[i]["yt"]  # [bcount, L, NCH+1, C] fp16
        gb = i * bcount
        out[gb : gb + bcount, 0:2] = ytc[:, 0:2, NCH]
        body = ytc[:, :, 0:NBLK].transpose(0, 2, 1, 3).reshape(bcount, NBLK * L, C)
        out[gb : gb + bcount, 2 : 2 + NBLK * L] = body
        out[gb : gb + bcount, 2 + NBLK * L :] = ytc[:, 0:TAIL, NBLK]
    return out


def _run(x, alpha, beta, trace=False, **kw):
    x = np.asarray(x, dtype=np.float32)
    assert x.shape == (B, T, C), x.shape
    in_maps = make_in_maps(x, alpha, beta)
    nc = _get_module()
    res = run_bass_kernel_spmd(nc, in_maps, list(range(N_CORES)), trace=trace, **kw)
    return _gather(res), res


def kernel(x, alpha, beta):
    return _run(x, alpha, beta)[0]
